# revision 44
# baseline (speedup 1.0000x reference)
"""Multi-head causal attention (B=2, S=2048, D=1024, H=16) on 8 TRN2 NeuronCores.

Sharding: tensor-parallel over heads x data-parallel over batch.
Core c handles batch b = c // 4 and head group g = c % 4 (heads 4g..4g+3),
i.e. a [2048, 256] slice of the output.

v2 design notes (vs the fp32r baseline at ~222us):
  - All matmul data is bf16 (host-converted): 1 cycle/row at any moving
    width, half the DMA and SBUF traffic. PSUM accumulation stays fp32.
  - Softmax normalization moved to the host: the kernel emits raw PV
    accumulations plus denominators (V' carries a ones column per head),
    removing the Ln/Exp reciprocal chain (~23us of ScalarE), the
    broadcast matmuls and the normalize multiplies.
  - The projection matmuls (pure PE work) are interleaved into the
    attention sweep as filler so the PE never idles: the HAM clock gate
    re-throttles the PE to 1.2 GHz after ~3.4us of idleness, which is
    what made the baseline's ScalarE-bound attention phase double the
    cost of everything on the tensor engine.
  - Attention windows are software-pipelined one deep: PE program order
    is [scores(w), PV(w-1), filler] so the PE never sits behind exp(w).
  - Head pairs share one [128, 2*512] score tile (two K=64 matmuls to
    distinct PE row groups via tile_position) and a single strided exp.
"""

import os
import sys

import numpy as np

for _p in ("/opt/trn_rl_repo", "/root/.axon_site/_ro/trn_rl_repo"):
    if os.path.isdir(_p) and _p not in sys.path:
        sys.path.insert(0, _p)

import ml_dtypes

BF = ml_dtypes.bfloat16

B, S, D, H = 2, 2048, 1024, 16
N_CORES = 8
HEADS_PER_CORE = 4
DH = D // H  # 64
DCORE = HEADS_PER_CORE * DH  # 256
KT = D // 128  # 8 contraction tiles for the projections
ST = S // 128  # 16 sequence tiles
QB = 512  # q block width
NJ = S // QB  # 4 q blocks
NEG = -1.0e30
OUTR = HEADS_PER_CORE * (DH + 1)  # 260 rows: per head 64 PV rows + 1 denom

_CACHE = {}


def _split_multi_waits(nc, max_waits=1):
    """This walrus build rejects instructions carrying more than one
    semaphore wait; hoist extras onto preceding NoOps on the same engine."""
    import bass_rust as _br

    n = 0
    for fn in nc.m.functions:
        for bb in fn.blocks:
            insts = list(bb.instructions)
            new = []
            changed = False
            for inst in insts:
                si = getattr(inst, "sync_info", None)
                ow = list(si.on_wait) if si is not None else []
                if len(ow) > max_waits:
                    changed = True
                    for w in ow[:-max_waits]:
                        n += 1
                        new.append(
                            _br.InstNoOp(
                                name=f"I-ws{n}",
                                engine=inst.engine,
                                ins=[],
                                outs=[],
                                sync_info=_br.SyncInfo(on_wait=[w], on_update=[]),
                            )
                        )
                    si.on_wait = ow[-max_waits:]
                    inst.sync_info = si
                new.append(inst)
            if changed:
                bb.instructions = new


def build_module(repeat=1, hw_loop=False, dbg=False):
    import contextlib

    import concourse.bass as bass
    import concourse.mybir as mybir
    from concourse.tile import TileContext

    F32 = mybir.dt.float32
    BF16 = mybir.dt.bfloat16
    AF = mybir.ActivationFunctionType

    nc = bass.Bass("TRN2", target_bir_lowering=False, debug=False, num_devices=N_CORES)

    xT_in = nc.declare_dram_parameter("xT", [D, S], BF16, isOutput=False)
    wq_in = nc.declare_dram_parameter("wq", [D, DCORE], BF16, isOutput=False)
    wk_in = nc.declare_dram_parameter("wk", [D, DCORE], BF16, isOutput=False)
    wv_in = nc.declare_dram_parameter("wv", [D, DCORE], BF16, isOutput=False)
    bq_in = nc.declare_dram_parameter("bq", [DCORE], F32, isOutput=False)
    bk_in = nc.declare_dram_parameter("bk", [DCORE], F32, isOutput=False)
    tri_in = nc.declare_dram_parameter("tri", [128, 256], BF16, isOutput=False)
    outR = nc.declare_dram_parameter("outR", [OUTR, S], F32, isOutput=True)
    if dbg:
        qdbg = nc.declare_dram_parameter("qdbg", [128, S], BF16, isOutput=True)
        kdbg = nc.declare_dram_parameter("kdbg", [128, S], BF16, isOutput=True)
        vdbg = nc.declare_dram_parameter("vdbg", [128, 4 * 65], BF16, isOutput=True)
        pdbg = nc.declare_dram_parameter("pdbg", [128, 2 * QB], BF16, isOutput=True)
        sdbg = nc.declare_dram_parameter("sdbg", [128, 2 * QB], F32, isOutput=True)

    with TileContext(nc) as tc:
        with (
            tc.tile_pool(name="persist", bufs=1) as pp,
            tc.tile_pool(name="work", bufs=3) as wp,
            tc.tile_pool(name="outp", bufs=3) as op,
            tc.tile_pool(name="sps_ps", bufs=2, space="PSUM") as sps_ps,
            tc.tile_pool(name="pv_ps", bufs=2, space="PSUM") as pv_ps,
            tc.tile_pool(name="proj_ps", bufs=2, space="PSUM") as proj_ps,
        ):
            # ---- constant / persistent tiles -------------------------------
            # DMA issue serializes at ~650ns per descriptor on the issuing
            # engine's queue, so spread input loads across sync/vector/gpsimd
            # and keep the lead-in-critical ones (wq, wk, x block 0) first.
            onesr = pp.tile([1, 128], BF16, tag="onesr")  # K=1 matmul lhsT
            nc.vector.memset(onesr[:], 1.0)

            # weights as one DMA per matrix: wX_all[:, 256k:256(k+1)] is the
            # [128, 256] k-th contraction tile
            wq_all = pp.tile([128, KT * DCORE], BF16, tag="wq_all")
            wk_all = pp.tile([128, KT * DCORE], BF16, tag="wk_all")
            wv_all = pp.tile([128, KT * DCORE], BF16, tag="wv_all")
            for t, src in ((wq_all, wq_in), (wk_all, wk_in), (wv_all, wv_in)):
                nc.gpsimd.dma_start(
                    t[:].rearrange("p (kt c) -> p kt c", kt=KT),
                    src[:].rearrange("(kt p) c -> p kt c", p=128),
                )


            # 0/1 causal triangle, bf16: multiplied into the probs AFTER the
            # exp (keeps the DVE off the scores->exp critical path; bv is
            # likewise folded in on the host since softmax weights sum to 1)
            trid = pp.tile([128, 256], BF16, tag="trid")
            nc.gpsimd.dma_start(trid[:], tri_in[:])
            bqc = pp.tile([128, 2], F32, tag="bqc")
            nc.gpsimd.dma_start(bqc[:], bq_in[:].rearrange("(m p) -> p m", p=128))
            bkc = pp.tile([128, 2], F32, tag="bkc")
            nc.gpsimd.dma_start(bkc[:], bk_in[:].rearrange("(m p) -> p m", p=128))

            # ---- warmup during the DMA window: dummy matmuls ramp the PE
            # HAM clock gate toward 2.4 GHz, one exp pulls the activation
            # table load off the critical path --------------------------------
            warm_ps = sps_ps.tile([128, 2 * QB], F32, tag="sps", name="warm_ps")
            for _w in range(20):
                nc.tensor.matmul(
                    warm_ps[:, 0:128], onesr[:], onesr[:], start=True, stop=True
                )

            # x^T tiles, loaded in [128, QB] slices n-major so the first
            # projection blocks can start after ~1/8 of x has landed.
            # block 0 is split across two queues; blocks 2-3 go behind the
            # weights on the gpsimd queue.
            xt = [pp.tile([128, S], BF16, tag=f"xt{k}", name=f"xt{k}") for k in range(KT)]

            def x_dma(eng, k, n):
                eng.dma_start(
                    xt[k][:, QB * n : QB * (n + 1)],
                    xT_in[128 * k : 128 * (k + 1), QB * n : QB * (n + 1)],
                )

            for k in range(4):
                x_dma(nc.sync, k, 0)
            for k in range(4, KT):
                x_dma(nc.scalar, k, 0)
            for k in range(KT):
                x_dma(nc.sync, k, 1)
            for n in (2, 3):
                for k in range(KT):
                    x_dma(nc.gpsimd, k, n)

            # exp table load after the scalar-queue DMAs, still ahead of the
            # first window's exp
            warm_o = wp.tile([1, 128], F32, tag="warm", name="warm_o")
            nc.scalar.activation(warm_o[:], onesr[:], AF.Exp)

            qT = [pp.tile([128, S], BF16, tag=f"qT{m}", name=f"qT{m}") for m in range(2)]
            kTt = [pp.tile([128, S], BF16, tag=f"kT{m}", name=f"kT{m}") for m in range(2)]
            vp = [
                pp.tile([128, HEADS_PER_CORE * 65], BF16, tag=f"vp{s}", name=f"vp{s}")
                for s in range(ST)
            ]
            for s in range(ST):
                nc.vector.memset(
                    vp[s][:].rearrange("p (h c) -> p h c", c=65)[:, :, 64:65], 1.0
                )

            # ---- projection unit emitters ----------------------------------
            def emit_qku(pk, pair, nb):
                # one [128, 512] block of Q^T or K^T for one head pair
                w_all = wq_all if pk == 0 else wk_all
                dst = qT[pair] if pk == 0 else kTt[pair]
                bias = bqc if pk == 0 else bkc
                acc = proj_ps.tile([128, QB], F32, tag="proj", name="qk_acc")
                for k in range(KT):
                    off = DCORE * k + 128 * pair
                    nc.tensor.matmul(
                        acc[:],
                        w_all[:, off : off + 128],
                        xt[k][:, QB * nb : QB * (nb + 1)],
                        start=(k == 0),
                        stop=(k == KT - 1),
                    )
                nc.vector.tensor_scalar_add(
                    dst[:, QB * nb : QB * (nb + 1)], acc[:], bias[:, pair : pair + 1]
                )

            def emit_vu(s):
                # V' tile s: [128 seq, 256 d], drained to bf16 with the
                # per-head ones columns preserved (bv is added on the host)
                acc = proj_ps.tile([128, QB], F32, tag="proj", name="v_acc")
                for k in range(KT):
                    nc.tensor.matmul(
                        acc[:, 0:DCORE],
                        xt[k][:, 128 * s : 128 * (s + 1)],
                        wv_all[:, DCORE * k : DCORE * (k + 1)],
                        start=(k == 0),
                        stop=(k == KT - 1),
                    )
                nc.vector.tensor_copy(
                    vp[s][:].rearrange("p (h c) -> p h c", c=65)[:, :, 0:64],
                    acc[:, 0:DCORE].rearrange("p (h c) -> p h c", c=64),
                )

            # Filler units: (need_by_window_index, kind, args), eagerly
            # spread so the PE load is even.  Need-by indices assume the
            # group order:
            # P0j0:0-3  P0j1:4-11  P1j0:12-15  P1j1:16-23
            # P0j2:24-35  P1j2:36-47  P0j3:48-63  P1j3:64-79
            # qku args = (pk, pair, nb) with pk 0=q, 1=k.
            # The two trailing qku units write NEXT iteration's first q/k
            # blocks (safe: last reader of kT[0] nb0 is window 51).
            units = [
                (0, "vu", (0,)),
                (1, "vu", (1,)),
                (2, "vu", (2,)),
                (3, "vu", (3,)),
                (4, "qku", (0, 0, 1)),
                (5, "qku", (1, 1, 0)),
                (6, "qku", (0, 1, 0)),
                (7, "qku", (1, 0, 1)),
                (8, "vu", (4,)),
                (9, "vu", (5,)),
                (10, "vu", (6,)),
                (11, "vu", (7,)),
                (13, "qku", (0, 1, 1)),
                (17, "qku", (1, 1, 1)),
                (21, "qku", (0, 0, 2)),
                (26, "qku", (1, 0, 2)),
                (28, "vu", (8,)),
                (30, "vu", (9,)),
                (32, "vu", (10,)),
                (33, "vu", (11,)),
                (34, "qku", (0, 1, 2)),
                (40, "qku", (1, 1, 2)),
                (45, "qku", (0, 0, 3)),
                (54, "qku", (1, 0, 3)),
                (56, "vu", (12,)),
                (58, "vu", (13,)),
                (60, "vu", (14,)),
                (61, "vu", (15,)),
                (62, "qku", (0, 1, 3)),
                (70, "qku", (1, 1, 3)),
                (72, "qku", (0, 0, 0)),
                (74, "qku", (1, 0, 0)),
            ]

            def run_unit(u):
                if u[1] == "vu":
                    emit_vu(*u[2])
                else:
                    emit_qku(*u[2])

            # ---- prologue: iteration 0's first q/k blocks (later
            # iterations get them from the previous sweep's tail) ------------
            emit_qku(0, 0, 0)
            emit_qku(1, 0, 0)

            def emit_body():
                # ---- attention sweep ---------------------------------------
                groups = [
                    (0, 0), (0, 1), (1, 0), (1, 1),
                    (0, 2), (1, 2), (0, 3), (1, 3),
                ]

                pending_pv = [None]
                uidx = [0]
                widx = [0]

                def emit_window(pair, j, t, accs):
                    i = t - 4 * j
                    qoff = 128 * max(i, 0)
                    qwin = slice(QB * j + qoff, QB * (j + 1))
                    ktile = slice(128 * t, 128 * (t + 1))
                    sps = sps_ps.tile([128, 2 * QB], F32, tag="sps", name="sps")
                    nc.tensor.matmul(
                        sps[:, qoff:QB],
                        kTt[pair][0:64, ktile],
                        qT[pair][0:64, qwin],
                        start=True,
                        stop=True,
                        tile_position=(0, 0),
                    )
                    nc.tensor.matmul(
                        sps[:, QB + qoff : 2 * QB],
                        kTt[pair][64:128, ktile],
                        qT[pair][64:128, qwin],
                        start=True,
                        stop=True,
                        tile_position=(64, 0),
                    )
                    spsv = sps[:].rearrange("p (two c) -> p two c", two=2)
                    pt = wp.tile([128, 2 * QB], BF16, tag="pt", name="pt")
                    ptv = pt[:].rearrange("p (two c) -> p two c", two=2)
                    if dbg and pair == 0 and j == 0 and t == 0:
                        sd = op.tile([128, 2 * QB], F32, tag="sd", name="sd")
                        nc.vector.tensor_copy(sd[:], sps[:])
                        nc.sync.dma_start(sdbg[:], sd[:])
                    nc.scalar.activation(
                        ptv[:, :, qoff:QB],
                        spsv[:, :, qoff:QB],
                        AF.Exp,
                        scale=float(1.0 / np.sqrt(DH)),
                    )
                    if i >= 0:
                        # zero the strictly-upper triangle of the diagonal
                        # block in the probs (bf16 2x multiply, off the
                        # scores->exp chain)
                        nc.vector.tensor_mul(
                            ptv[:, :, qoff : qoff + 128],
                            ptv[:, :, qoff : qoff + 128],
                            trid[:].rearrange("p (two c) -> p two c", two=2),
                        )
                    if dbg and pair == 0 and j == 0 and t == 0:
                        nc.sync.dma_start(pdbg[:], pt[:])

                    def pv():
                        for h_loc, off in ((0, 0), (1, QB)):
                            h = 2 * pair + h_loc
                            nc.tensor.matmul(
                                accs[h_loc][0:65, qoff:QB],
                                vp[t][:, 65 * h : 65 * h + 65],
                                pt[:, off + qoff : off + QB],
                                start=(t == 0),
                                stop=(t == 4 * j + 3),
                            )
                        if t == 4 * j + 3:
                            for h_loc in (0, 1):
                                h = 2 * pair + h_loc
                                o = op.tile([65, QB], F32, tag="out", name="o")
                                nc.vector.tensor_copy(o[:], accs[h_loc][0:65, :])
                                nc.sync.dma_start(
                                    outR[65 * h : 65 * h + 65, QB * j : QB * (j + 1)],
                                    o[:],
                                )

                    return pv

                for pair, j in groups:
                    accs = [
                        pv_ps.tile([128, QB], F32, tag="pv", name=f"acc{pair}_{j}_{hl}")
                        for hl in (0, 1)
                    ]
                    for t in range(4 * j + 4):
                        # demand-driven projection filler (before the window
                        # that first reads its output)
                        while uidx[0] < len(units) and units[uidx[0]][0] <= widx[0]:
                            run_unit(units[uidx[0]])
                            uidx[0] += 1
                        pv = emit_window(pair, j, t, accs)
                        if pending_pv[0] is not None:
                            pending_pv[0]()
                        pending_pv[0] = pv
                        widx[0] += 1
                if pending_pv[0] is not None:
                    pending_pv[0]()
                if dbg:
                    nc.sync.dma_start(qdbg[:], qT[0][:])
                    nc.sync.dma_start(kdbg[:], kTt[0][:])
                    nc.sync.dma_start(vdbg[:], vp[0][:])

            # The For_i back edge is an all-engine barrier costing ~10us plus
            # a HAM cold restart; unroll UNROLL bodies per loop iteration so
            # it amortizes.
            UNROLL = 8
            if hw_loop and repeat > UNROLL:
                n_loop = repeat // UNROLL
                with tc.For_i(0, n_loop, 1):
                    for _u in range(UNROLL):
                        emit_body()
                for _u in range(repeat - n_loop * UNROLL):
                    emit_body()
            else:
                for _u in range(repeat):
                    emit_body()

    _split_multi_waits(nc)
    return nc


def _get_runner():
    if "nc" not in _CACHE:
        _CACHE["nc"] = build_module()
    return _CACHE["nc"]


def _make_in_maps(x, Wq, bq, Wk, bk, Wv, bv):
    x = np.asarray(x, dtype=np.float32)
    Wq = np.asarray(Wq, dtype=np.float32)
    Wk = np.asarray(Wk, dtype=np.float32)
    Wv = np.asarray(Wv, dtype=np.float32)
    bq = np.asarray(bq, dtype=np.float32)
    bk = np.asarray(bk, dtype=np.float32)
    bv = np.asarray(bv, dtype=np.float32)

    kp = np.arange(128)[:, None]
    qf = np.arange(128)[None, :]
    tri = np.where(kp <= qf, 1.0, 0.0).astype(BF)
    trid = np.concatenate([tri, tri], axis=1)

    xTs = [np.ascontiguousarray(x[b].T).astype(BF) for b in range(B)]
    in_maps = []
    for c in range(N_CORES):
        b = c // 4
        g = c % 4
        sl = slice(DCORE * g, DCORE * (g + 1))
        in_maps.append(
            {
                "xT": xTs[b],
                "wq": np.ascontiguousarray(Wq[:, sl]).astype(BF),
                "wk": np.ascontiguousarray(Wk[:, sl]).astype(BF),
                "wv": np.ascontiguousarray(Wv[:, sl]).astype(BF),
                "bq": np.ascontiguousarray(bq[sl]),
                "bk": np.ascontiguousarray(bk[sl]),
                "tri": trid,
            }
        )
    return in_maps


def kernel(x, Wq, bq, Wk, bk, Wv, bv):
    from concourse.bass_utils import run_bass_kernel_spmd

    nc = _get_runner()
    in_maps = _make_in_maps(x, Wq, bq, Wk, bk, Wv, bv)
    res = run_bass_kernel_spmd(nc, in_maps, list(range(N_CORES)))
    out = np.empty((B, S, D), dtype=np.float32)
    for c in range(N_CORES):
        b = c // 4
        g = c % 4
        raw = res.results[c]["outR"]  # [260, 2048]
        for h in range(HEADS_PER_CORE):
            num = raw[65 * h : 65 * h + 64, :]
            den = raw[65 * h + 64, :]
            dlo = DCORE * g + 64 * h
            out[b, :, dlo : dlo + 64] = (num / den).T + np.asarray(bv, np.float32)[dlo : dlo + 64]
    return out


# revision 45
# speedup vs baseline: 1.0536x; 1.0536x over previous
"""Multi-head causal attention (B=2, S=2048, D=1024, H=16) on 8 TRN2 NeuronCores.

Sharding: tensor-parallel over heads x data-parallel over batch.
Core c handles batch b = c // 4 and head group g = c % 4 (heads 4g..4g+3),
i.e. a [2048, 256] slice of the output.

v2 design notes (vs the fp32r baseline at ~222us):
  - All matmul data is bf16 (host-converted): 1 cycle/row at any moving
    width, half the DMA and SBUF traffic. PSUM accumulation stays fp32.
  - Softmax normalization moved to the host: the kernel emits raw PV
    accumulations plus denominators (V' carries a ones column per head),
    removing the Ln/Exp reciprocal chain (~23us of ScalarE), the
    broadcast matmuls and the normalize multiplies.
  - The projection matmuls (pure PE work) are interleaved into the
    attention sweep as filler so the PE never idles: the HAM clock gate
    re-throttles the PE to 1.2 GHz after ~3.4us of idleness, which is
    what made the baseline's ScalarE-bound attention phase double the
    cost of everything on the tensor engine.
  - Attention windows are software-pipelined one deep: PE program order
    is [scores(w), PV(w-1), filler] so the PE never sits behind exp(w).
  - Head pairs share one [128, 2*512] score tile (two K=64 matmuls to
    distinct PE row groups via tile_position) and a single strided exp.
"""

import os
import sys

import numpy as np

for _p in ("/opt/trn_rl_repo", "/root/.axon_site/_ro/trn_rl_repo"):
    if os.path.isdir(_p) and _p not in sys.path:
        sys.path.insert(0, _p)

import ml_dtypes

BF = ml_dtypes.bfloat16

B, S, D, H = 2, 2048, 1024, 16
N_CORES = 8
HEADS_PER_CORE = 4
DH = D // H  # 64
DCORE = HEADS_PER_CORE * DH  # 256
KT = D // 128  # 8 contraction tiles for the projections
ST = S // 128  # 16 sequence tiles
QB = 512  # q block width
NJ = S // QB  # 4 q blocks
NEG = -1.0e30
OUTR = HEADS_PER_CORE * (DH + 1)  # 260 rows: per head 64 PV rows + 1 denom

_CACHE = {}


def _split_multi_waits(nc, max_waits=1):
    """This walrus build rejects instructions carrying more than one
    semaphore wait; hoist extras onto preceding NoOps on the same engine."""
    import bass_rust as _br

    n = 0
    for fn in nc.m.functions:
        for bb in fn.blocks:
            insts = list(bb.instructions)
            new = []
            changed = False
            for inst in insts:
                si = getattr(inst, "sync_info", None)
                ow = list(si.on_wait) if si is not None else []
                if len(ow) > max_waits:
                    changed = True
                    for w in ow[:-max_waits]:
                        n += 1
                        new.append(
                            _br.InstNoOp(
                                name=f"I-ws{n}",
                                engine=inst.engine,
                                ins=[],
                                outs=[],
                                sync_info=_br.SyncInfo(on_wait=[w], on_update=[]),
                            )
                        )
                    si.on_wait = ow[-max_waits:]
                    inst.sync_info = si
                new.append(inst)
            if changed:
                bb.instructions = new


def build_module(repeat=1, hw_loop=False, dbg=False):
    import contextlib

    import concourse.bass as bass
    import concourse.mybir as mybir
    from concourse.tile import TileContext

    F32 = mybir.dt.float32
    BF16 = mybir.dt.bfloat16
    AF = mybir.ActivationFunctionType

    nc = bass.Bass("TRN2", target_bir_lowering=False, debug=False, num_devices=N_CORES)

    xT_in = nc.declare_dram_parameter("xT", [D, S], BF16, isOutput=False)
    wq_in = nc.declare_dram_parameter("wq", [D, DCORE], BF16, isOutput=False)
    wk_in = nc.declare_dram_parameter("wk", [D, DCORE], BF16, isOutput=False)
    wv_in = nc.declare_dram_parameter("wv", [D, DCORE], BF16, isOutput=False)
    bq_in = nc.declare_dram_parameter("bq", [DCORE], F32, isOutput=False)
    bk_in = nc.declare_dram_parameter("bk", [DCORE], F32, isOutput=False)
    bv_in = nc.declare_dram_parameter("bv", [DCORE], BF16, isOutput=False)
    tri_in = nc.declare_dram_parameter("tri", [128, 256], F32, isOutput=False)
    outR = nc.declare_dram_parameter("outR", [OUTR, S], F32, isOutput=True)
    if dbg:
        qdbg = nc.declare_dram_parameter("qdbg", [128, S], BF16, isOutput=True)
        kdbg = nc.declare_dram_parameter("kdbg", [128, S], BF16, isOutput=True)
        vdbg = nc.declare_dram_parameter("vdbg", [128, 4 * 65], BF16, isOutput=True)
        pdbg = nc.declare_dram_parameter("pdbg", [128, 2 * QB], BF16, isOutput=True)
        sdbg = nc.declare_dram_parameter("sdbg", [128, 2 * QB], F32, isOutput=True)

    with TileContext(nc) as tc:
        with (
            tc.tile_pool(name="persist", bufs=1) as pp,
            tc.tile_pool(name="work", bufs=3) as wp,
            tc.tile_pool(name="outp", bufs=3) as op,
            tc.tile_pool(name="sps_ps", bufs=2, space="PSUM") as sps_ps,
            tc.tile_pool(name="pv_ps", bufs=2, space="PSUM") as pv_ps,
            tc.tile_pool(name="proj_ps", bufs=2, space="PSUM") as proj_ps,
        ):
            # ---- constant / persistent tiles -------------------------------
            # DMA issue serializes at ~650ns per descriptor on the issuing
            # engine's queue, so spread input loads across sync/vector/gpsimd
            # and keep the lead-in-critical ones (wq, wk, x block 0) first.
            onesr = pp.tile([1, 128], BF16, tag="onesr")  # K=1 matmul lhsT
            nc.vector.memset(onesr[:], 1.0)

            # weights as one DMA per matrix: wX_all[:, 256k:256(k+1)] is the
            # [128, 256] k-th contraction tile
            wq_all = pp.tile([128, KT * DCORE], BF16, tag="wq_all")
            wk_all = pp.tile([128, KT * DCORE], BF16, tag="wk_all")
            wv_all = pp.tile([128, KT * DCORE], BF16, tag="wv_all")
            for t, src in ((wq_all, wq_in), (wk_all, wk_in), (wv_all, wv_in)):
                nc.gpsimd.dma_start(
                    t[:].rearrange("p (kt c) -> p kt c", kt=KT),
                    src[:].rearrange("(kt p) c -> p kt c", p=128),
                )


            trid = pp.tile([128, 256], F32, tag="trid")
            nc.gpsimd.dma_start(trid[:], tri_in[:])
            bvrow = pp.tile([1, DCORE], BF16, tag="bvrow")
            nc.gpsimd.dma_start(bvrow[:], bv_in[:].rearrange("(a b) -> a b", a=1))
            bqc = pp.tile([128, 2], F32, tag="bqc")
            nc.gpsimd.dma_start(bqc[:], bq_in[:].rearrange("(m p) -> p m", p=128))
            bkc = pp.tile([128, 2], F32, tag="bkc")
            nc.gpsimd.dma_start(bkc[:], bk_in[:].rearrange("(m p) -> p m", p=128))

            # ---- warmup during the DMA window: dummy matmuls ramp the PE
            # HAM clock gate toward 2.4 GHz, one exp pulls the activation
            # table load off the critical path --------------------------------
            warm_ps = sps_ps.tile([128, 2 * QB], F32, tag="sps", name="warm_ps")
            for _w in range(20):
                nc.tensor.matmul(
                    warm_ps[:, 0:128], onesr[:], onesr[:], start=True, stop=True
                )

            # x^T tiles, loaded in [128, QB] slices n-major so the first
            # projection blocks can start after ~1/8 of x has landed.
            # block 0 is split across two queues; blocks 2-3 go behind the
            # weights on the gpsimd queue.
            xt = [pp.tile([128, S], BF16, tag=f"xt{k}", name=f"xt{k}") for k in range(KT)]

            def x_dma(eng, k, n):
                eng.dma_start(
                    xt[k][:, QB * n : QB * (n + 1)],
                    xT_in[128 * k : 128 * (k + 1), QB * n : QB * (n + 1)],
                )

            for k in range(4):
                x_dma(nc.sync, k, 0)
            for k in range(4, KT):
                x_dma(nc.scalar, k, 0)
            for k in range(KT):
                x_dma(nc.sync, k, 1)
            for n in (2, 3):
                for k in range(KT):
                    x_dma(nc.gpsimd, k, n)

            # exp table load after the scalar-queue DMAs, still ahead of the
            # first window's exp
            warm_o = wp.tile([1, 128], F32, tag="warm", name="warm_o")
            nc.scalar.activation(warm_o[:], onesr[:], AF.Exp)

            qT = [pp.tile([128, S], BF16, tag=f"qT{m}", name=f"qT{m}") for m in range(2)]
            kTt = [pp.tile([128, S], BF16, tag=f"kT{m}", name=f"kT{m}") for m in range(2)]
            vp = [
                pp.tile([128, HEADS_PER_CORE * 65], BF16, tag=f"vp{s}", name=f"vp{s}")
                for s in range(ST)
            ]
            for s in range(ST):
                nc.vector.memset(
                    vp[s][:].rearrange("p (h c) -> p h c", c=65)[:, :, 64:65], 1.0
                )

            # ---- projection unit emitters ----------------------------------
            def emit_qku(pk, pair, nb):
                # one [128, 512] block of Q^T or K^T for one head pair
                w_all = wq_all if pk == 0 else wk_all
                dst = qT[pair] if pk == 0 else kTt[pair]
                bias = bqc if pk == 0 else bkc
                acc = proj_ps.tile([128, QB], F32, tag="proj", name="qk_acc")
                for k in range(KT):
                    off = DCORE * k + 128 * pair
                    nc.tensor.matmul(
                        acc[:],
                        w_all[:, off : off + 128],
                        xt[k][:, QB * nb : QB * (nb + 1)],
                        start=(k == 0),
                        stop=(k == KT - 1),
                    )
                nc.vector.tensor_scalar_add(
                    dst[:, QB * nb : QB * (nb + 1)], acc[:], bias[:, pair : pair + 1]
                )

            def emit_vu(s):
                # V' tile s: [128 seq, 256 d] + bias, drained to bf16 with
                # the per-head ones columns preserved
                acc = proj_ps.tile([128, QB], F32, tag="proj", name="v_acc")
                for k in range(KT):
                    nc.tensor.matmul(
                        acc[:, 0:DCORE],
                        xt[k][:, 128 * s : 128 * (s + 1)],
                        wv_all[:, DCORE * k : DCORE * (k + 1)],
                        start=(k == 0),
                        stop=False,
                    )
                nc.tensor.matmul(
                    acc[:, 0:DCORE], onesr[:], bvrow[:], start=False, stop=True
                )
                nc.vector.tensor_copy(
                    vp[s][:].rearrange("p (h c) -> p h c", c=65)[:, :, 0:64],
                    acc[:, 0:DCORE].rearrange("p (h c) -> p h c", c=64),
                )

            # Filler units: (need_by_window_index, kind, args), eagerly
            # spread so the PE load is even.  Need-by indices assume the
            # group order:
            # P0j0:0-3  P0j1:4-11  P1j0:12-15  P1j1:16-23
            # P0j2:24-35  P1j2:36-47  P0j3:48-63  P1j3:64-79
            # qku args = (pk, pair, nb) with pk 0=q, 1=k.
            # The two trailing qku units write NEXT iteration's first q/k
            # blocks (safe: last reader of kT[0] nb0 is window 51).
            units = [
                (0, "vu", (0,)),
                (1, "vu", (1,)),
                (2, "vu", (2,)),
                (3, "vu", (3,)),
                (4, "qku", (0, 0, 1)),
                (5, "qku", (1, 1, 0)),
                (6, "qku", (0, 1, 0)),
                (7, "qku", (1, 0, 1)),
                (8, "vu", (4,)),
                (9, "vu", (5,)),
                (10, "vu", (6,)),
                (11, "vu", (7,)),
                (13, "qku", (0, 1, 1)),
                (17, "qku", (1, 1, 1)),
                (21, "qku", (0, 0, 2)),
                (26, "qku", (1, 0, 2)),
                (28, "vu", (8,)),
                (30, "vu", (9,)),
                (32, "vu", (10,)),
                (33, "vu", (11,)),
                (34, "qku", (0, 1, 2)),
                (40, "qku", (1, 1, 2)),
                (45, "qku", (0, 0, 3)),
                (54, "qku", (1, 0, 3)),
                (56, "vu", (12,)),
                (58, "vu", (13,)),
                (60, "vu", (14,)),
                (61, "vu", (15,)),
                (62, "qku", (0, 1, 3)),
                (70, "qku", (1, 1, 3)),
                (72, "qku", (0, 0, 0)),
                (74, "qku", (1, 0, 0)),
            ]

            def run_unit(u):
                if u[1] == "vu":
                    emit_vu(*u[2])
                else:
                    emit_qku(*u[2])

            # ---- prologue: iteration 0's first q/k blocks (later
            # iterations get them from the previous sweep's tail) ------------
            emit_qku(0, 0, 0)
            emit_qku(1, 0, 0)

            def emit_body():
                # ---- attention sweep ---------------------------------------
                groups = [
                    (0, 0), (0, 1), (1, 0), (1, 1),
                    (0, 2), (1, 2), (0, 3), (1, 3),
                ]

                pending_pv = [None]
                uidx = [0]
                widx = [0]

                def emit_window(pair, j, t, accs):
                    i = t - 4 * j
                    qoff = 128 * max(i, 0)
                    qwin = slice(QB * j + qoff, QB * (j + 1))
                    ktile = slice(128 * t, 128 * (t + 1))
                    sps = sps_ps.tile([128, 2 * QB], F32, tag="sps", name="sps")
                    nc.tensor.matmul(
                        sps[:, qoff:QB],
                        kTt[pair][0:64, ktile],
                        qT[pair][0:64, qwin],
                        start=True,
                        stop=True,
                        tile_position=(0, 0),
                    )
                    nc.tensor.matmul(
                        sps[:, QB + qoff : 2 * QB],
                        kTt[pair][64:128, ktile],
                        qT[pair][64:128, qwin],
                        start=True,
                        stop=True,
                        tile_position=(64, 0),
                    )
                    spsv = sps[:].rearrange("p (two c) -> p two c", two=2)
                    if i >= 0:
                        nc.vector.tensor_add(
                            spsv[:, :, qoff : qoff + 128],
                            spsv[:, :, qoff : qoff + 128],
                            trid[:].rearrange("p (two c) -> p two c", two=2),
                        )
                    pt = wp.tile([128, 2 * QB], BF16, tag="pt", name="pt")
                    ptv = pt[:].rearrange("p (two c) -> p two c", two=2)
                    if dbg and pair == 0 and j == 0 and t == 0:
                        sd = op.tile([128, 2 * QB], F32, tag="sd", name="sd")
                        nc.vector.tensor_copy(sd[:], sps[:])
                        nc.sync.dma_start(sdbg[:], sd[:])
                    nc.scalar.activation(
                        ptv[:, :, qoff:QB],
                        spsv[:, :, qoff:QB],
                        AF.Exp,
                        scale=float(1.0 / np.sqrt(DH)),
                    )
                    if dbg and pair == 0 and j == 0 and t == 0:
                        nc.sync.dma_start(pdbg[:], pt[:])

                    def pv():
                        for h_loc, off in ((0, 0), (1, QB)):
                            h = 2 * pair + h_loc
                            nc.tensor.matmul(
                                accs[h_loc][0:65, qoff:QB],
                                vp[t][:, 65 * h : 65 * h + 65],
                                pt[:, off + qoff : off + QB],
                                start=(t == 0),
                                stop=(t == 4 * j + 3),
                            )
                        if t == 4 * j + 3:
                            for h_loc in (0, 1):
                                h = 2 * pair + h_loc
                                o = op.tile([65, QB], F32, tag="out", name="o")
                                nc.vector.tensor_copy(o[:], accs[h_loc][0:65, :])
                                nc.sync.dma_start(
                                    outR[65 * h : 65 * h + 65, QB * j : QB * (j + 1)],
                                    o[:],
                                )

                    return pv

                for pair, j in groups:
                    accs = [
                        pv_ps.tile([128, QB], F32, tag="pv", name=f"acc{pair}_{j}_{hl}")
                        for hl in (0, 1)
                    ]
                    for t in range(4 * j + 4):
                        # demand-driven projection filler (before the window
                        # that first reads its output)
                        while uidx[0] < len(units) and units[uidx[0]][0] <= widx[0]:
                            run_unit(units[uidx[0]])
                            uidx[0] += 1
                        pv = emit_window(pair, j, t, accs)
                        if pending_pv[0] is not None:
                            pending_pv[0]()
                        pending_pv[0] = pv
                        widx[0] += 1
                if pending_pv[0] is not None:
                    pending_pv[0]()
                if dbg:
                    nc.sync.dma_start(qdbg[:], qT[0][:])
                    nc.sync.dma_start(kdbg[:], kTt[0][:])
                    nc.sync.dma_start(vdbg[:], vp[0][:])

            # The For_i back edge is an all-engine barrier costing ~10us plus
            # a HAM cold restart; unroll UNROLL bodies per loop iteration so
            # it amortizes.
            UNROLL = 8
            if hw_loop and repeat > UNROLL:
                n_loop = repeat // UNROLL
                with tc.For_i(0, n_loop, 1):
                    for _u in range(UNROLL):
                        emit_body()
                for _u in range(repeat - n_loop * UNROLL):
                    emit_body()
            else:
                for _u in range(repeat):
                    emit_body()

    _split_multi_waits(nc)
    return nc


def _get_runner():
    if "nc" not in _CACHE:
        _CACHE["nc"] = build_module()
    return _CACHE["nc"]


def _make_in_maps(x, Wq, bq, Wk, bk, Wv, bv):
    x = np.asarray(x, dtype=np.float32)
    Wq = np.asarray(Wq, dtype=np.float32)
    Wk = np.asarray(Wk, dtype=np.float32)
    Wv = np.asarray(Wv, dtype=np.float32)
    bq = np.asarray(bq, dtype=np.float32)
    bk = np.asarray(bk, dtype=np.float32)
    bv = np.asarray(bv, dtype=np.float32)

    kp = np.arange(128)[:, None]
    qf = np.arange(128)[None, :]
    tri = np.where(kp <= qf, 0.0, NEG).astype(np.float32)
    trid = np.concatenate([tri, tri], axis=1)

    xTs = [np.ascontiguousarray(x[b].T).astype(BF) for b in range(B)]
    in_maps = []
    for c in range(N_CORES):
        b = c // 4
        g = c % 4
        sl = slice(DCORE * g, DCORE * (g + 1))
        in_maps.append(
            {
                "xT": xTs[b],
                "wq": np.ascontiguousarray(Wq[:, sl]).astype(BF),
                "wk": np.ascontiguousarray(Wk[:, sl]).astype(BF),
                "wv": np.ascontiguousarray(Wv[:, sl]).astype(BF),
                "bq": np.ascontiguousarray(bq[sl]),
                "bk": np.ascontiguousarray(bk[sl]),
                "bv": np.ascontiguousarray(bv[sl]).astype(BF),
                "tri": trid,
            }
        )
    return in_maps


def kernel(x, Wq, bq, Wk, bk, Wv, bv):
    from concourse.bass_utils import run_bass_kernel_spmd

    nc = _get_runner()
    in_maps = _make_in_maps(x, Wq, bq, Wk, bk, Wv, bv)
    res = run_bass_kernel_spmd(nc, in_maps, list(range(N_CORES)))
    out = np.empty((B, S, D), dtype=np.float32)
    for c in range(N_CORES):
        b = c // 4
        g = c % 4
        raw = res.results[c]["outR"]  # [260, 2048]
        for h in range(HEADS_PER_CORE):
            num = raw[65 * h : 65 * h + 64, :]
            den = raw[65 * h + 64, :]
            out[b, :, DCORE * g + 64 * h : DCORE * g + 64 * (h + 1)] = (num / den).T
    return out
